# revision 1
# baseline (speedup 1.0000x reference)
"""ConvCNP encoder kernel for 8x TRN2 NeuronCores.

Math: the reference computes, for a 128x128 uniform grid g=(xs[i], ys[j]) and
n=8192 data points X (2-D) with values psi(Y) = [1, Y0, Y1]:

    Gram[g, x] = exp(-0.5*||g - X[x]||^2)
    fm = Gram @ psi                  # (G, 3); column 0 == row-sum (denominator)
    out[c, j, i] = fm[(i, j), c], with c=1,2 normalized by column 0.

The squared distance is separable over the grid axes:

    Gram[(i,j), x] = A[i, x] * B[j, x]
      A[i, x] = exp(-0.5*(xs[i] - X0[x])^2)     B[j, x] = exp(-0.5*(ys[j] - X1[x])^2)

so, with Bc = B * psi_c (row-wise):  fm[(i,j), c] = sum_x Bc[j, x] * A[i, x].

Sharding: grid y-axis (j) across the 8 cores - 16 j-rows per core; X, Y
replicated. No cross-core communication. Per core:

    acc[(c,j), i] = sum over 64 x-chunks of  BfT_k^T @ AT_k      (PE, PSUM accum)
      AT_k  = exp(-0.5*(xs[i] - X0[x])^2)  in SBUF layout [x_part=128, i=128]
      BfT_k = [B | B*Y0 | B*Y1]            in SBUF layout [x_part=128, 48]

AT is produced by a fused custom DVE op  sq(Src0 - Src1)  over broadcast APs
(one 1x pass) followed by one big ACT Exp (scale=-0.5) per stripe. The Gram
factors are stored as fp16 (the fp32 argument keeps exp accuracy; fp16 values
feed the PE at 1 cycle/column instead of fp32's two 4-cycle passes).
"""

import numpy as np
from contextlib import ExitStack

N_AXIS = 128          # grid points per axis
NPTS = 8192           # data points
NCORES = 8
JS = N_AXIS // NCORES  # 16 grid-y rows per core
NCHUNK = NPTS // 128   # 64 contraction chunks of 128
STRIPE = 16            # chunks per elementwise stripe
NSTRIPES = NCHUNK // STRIPE
GRID_LO, GRID_HI = -2.0, 2.0

_CACHE = {}


def _register_sqdiff():
    """Register a fused (a-b)^2 custom DVE op (idempotent)."""
    from concourse import dve_ops
    from concourse.dve_spec import Spec, Src0, Src1, sq, lower
    from concourse.dve_uop import DveOpSpec

    name = "TENSOR_SQDIFF_X"
    for op in dve_ops.OPS:
        if op.name == name:
            return op
    spec = Spec(
        body=sq(Src0 - Src1),
        reference=lambda in0, in1, s0, s1, imm2: (in0.astype(np.float32) - in1) ** 2,
    )
    opcode = max(dve_ops._SUB_OPCODE_FOR_NAME.values()) + 1
    assert opcode < 0x20
    dve_ops._SUB_OPCODE_FOR_NAME[name] = opcode
    shas = {}
    for ver in ("v3", "v4"):
        s = DveOpSpec(name=name, opcode=opcode, uops=lower(spec, ver=ver), rd1_en=True)
        shas[ver] = s.sha(ver)
    op = dve_ops.DveOp(name, spec, subdim=False, uops_sha=shas)
    dve_ops.OPS.append(op)
    dve_ops.CUSTOM_DVE_SPECS[name] = spec
    return op


def _patch_walrus_flags():
    """Cap the compiler's semaphore file so the NEFF epilogue restores ~176
    semaphores instead of all 254 (the restore is ~40ns/sem/engine of pure
    tail latency). Idempotent."""
    import concourse.bass_utils as bu

    if getattr(bu.run_command, "_sem_cap_patched", False):
        return
    orig = bu.run_command

    def run_command_capped(argv, **kwargs):
        if argv and "walrus_driver" in str(argv[0]) and any(
                str(a).startswith("--neff-output-filename") for a in argv):
            argv = list(argv) + ["--max-sem-num=176"]
        return orig(argv, **kwargs)

    run_command_capped._sem_cap_patched = True
    bu.run_command = run_command_capped


def _build_program():
    import concourse.bacc as bacc
    import concourse.mybir as mybir
    import concourse.tile as tile

    _patch_walrus_flags()
    sqdiff = _register_sqdiff()

    f32 = mybir.dt.float32
    f16 = mybir.dt.float16
    nc = bacc.Bacc("TRN2", target_bir_lowering=False, debug=False, num_devices=NCORES,
                   enable_partition_id=False, monotonic_sem_count=0)

    # Packed inputs (fewer, earlier DMAs):
    #   bc [128, 80]  f32: x1t(0:64) | ysb(64:80)     -> unblocks the B chain
    #   ac [128, 192] f32: xsb(0:128) | x0t(128:192)  -> A stripes
    #   yc [128, 128] f16: y0t(0:64)  | y1t(64:128)   -> B*psi muls
    bc = nc.dram_tensor("bc", [128, 80], f32, kind="ExternalInput")
    ac = nc.dram_tensor("ac", [128, 192], f32, kind="ExternalInput")
    yc = nc.dram_tensor("yc", [128, 128], f16, kind="ExternalInput")
    out = nc.dram_tensor("out", [128, 3 * JS], f32, kind="ExternalOutput")

    with tile.TileContext(nc) as tc, ExitStack() as ctx:
        singles = ctx.enter_context(tc.tile_pool(name="singles", bufs=1))
        argp = ctx.enter_context(tc.tile_pool(name="argp", bufs=3))
        atp = ctx.enter_context(tc.tile_pool(name="atp", bufs=3))
        psum = ctx.enter_context(tc.tile_pool(name="psum", bufs=1, space="PSUM"))

        s_bc = singles.tile([128, 80], f32, tag="bc")
        nc.sync.dma_start(s_bc[:, :], bc[:, :])
        s_ac = singles.tile([128, 192], f32, tag="ac")
        nc.sync.dma_start(s_ac[:, :], ac[:, :])
        s_yc = singles.tile([128, 128], f16, tag="yc")
        nc.gpsimd.dma_start(s_yc[:, :], yc[:, :])

        x1t = s_bc[:, 0:64]
        ysb = s_bc[:, 64:80]
        xsb = s_ac[:, 0:128]
        x0t = s_ac[:, 128:192]

        # ---- B side: BfT[x_p, k, 0:48] = [B | B*Y0 | B*Y1], all 64 chunks ----
        s_bsq = singles.tile([128, NCHUNK, JS], f32, tag="bsq")
        s_bf = singles.tile([128, NCHUNK, 3 * JS], f16, tag="bf")

        nc.vector._custom_dve(
            sqdiff,
            out=s_bsq[:, :, :],
            in0=ysb.unsqueeze(1).broadcast_to([128, NCHUNK, JS]),
            in1=x1t.unsqueeze(2).broadcast_to([128, NCHUNK, JS]),
        )
        nc.scalar.activation(
            s_bf[:, :, 0:JS], s_bsq[:, :, :],
            mybir.ActivationFunctionType.Exp, scale=-0.5,
        )

        # ---- A side + matmul, striped; B*psi muls slotted after stripe 0 ----
        # acc[i, (c, j)]: lhsT = AT chunk (128 fp16 weight cols), rhs = BfT
        # chunk [128, 48]. (c, j) on the free axis makes the normalization a
        # plain broadcast-AP multiply.
        acc = psum.tile([128, 3 * JS], f32, tag="acc")
        stripe_sizes = [16, 16, 16, 8, 8]
        assert sum(stripe_sizes) == NCHUNK
        k0 = 0
        for s, width in enumerate(stripe_sizes):
            arg = argp.tile([128, STRIPE, 128], f32, tag="arg", name="arg")[:, 0:width, :]
            nc.vector._custom_dve(
                sqdiff,
                out=arg[:, :, :],
                in0=xsb.unsqueeze(1).broadcast_to([128, width, 128]),
                in1=x0t[:, k0:k0 + width].unsqueeze(2).broadcast_to(
                    [128, width, 128]),
            )
            at = atp.tile([128, STRIPE, 128], f16, tag="at", name="at")[:, 0:width, :]
            nc.scalar.activation(
                at[:, :, :], arg[:, :, :],
                mybir.ActivationFunctionType.Exp, scale=-0.5,
            )
            if s == 0:
                # B*Y0, B*Y1 (DVE, 1x with the broadcast psi operand) — after
                # stripe 0 so the DVE isn't stalled on the B exp, before the
                # first matmul needs the full 48-column BfT.
                for c in range(2):
                    nc.vector.tensor_tensor(
                        s_bf[:, :, (c + 1) * JS:(c + 2) * JS], s_bf[:, :, 0:JS],
                        s_yc[:, c * NCHUNK:(c + 1) * NCHUNK].unsqueeze(2)
                            .broadcast_to([128, NCHUNK, JS]),
                        mybir.AluOpType.mult,
                    )
            for k in range(width):
                nc.tensor.matmul(
                    acc[:, :],
                    at[:, k, :],         # stationary lhsT: [128, 128] fp16
                    s_bf[:, k0 + k, :],  # moving rhs: [128, 48] fp16
                    start=(k0 + k == 0),
                    stop=(k0 + k == NCHUNK - 1),
                )
            k0 += width

        # ---- epilogue: normalize columns 1,2 by column 0 (the row-sum) ----
        s_rec = singles.tile([128, JS], f32, tag="rec")
        nc.vector.reciprocal_approx_fast(s_rec[:, :], acc[:, 0:JS])
        s_out = singles.tile([128, 3 * JS], f32, tag="outt")
        nc.vector.tensor_copy(s_out[:, 0:JS], acc[:, 0:JS])
        nc.vector.tensor_tensor(
            s_out[:, JS:3 * JS].rearrange("p (c j) -> p c j", c=2),
            acc[:, JS:3 * JS].rearrange("p (c j) -> p c j", c=2),
            s_rec[:, :].unsqueeze(1).broadcast_to([128, 2, JS]),
            mybir.AluOpType.mult,
        )
        nc.sync.dma_start(out[:, :], s_out[:, :])

    nc.finalize()
    return nc


def _get_program():
    if "nc" not in _CACHE:
        _CACHE["nc"] = _build_program()
    return _CACHE["nc"]


def _host_inputs(X, Y):
    """Build the per-core input maps (layout prep only)."""
    X = np.ascontiguousarray(np.asarray(X, dtype=np.float32))
    Y = np.ascontiguousarray(np.asarray(Y, dtype=np.float32))
    xs = np.linspace(GRID_LO, GRID_HI, N_AXIS, dtype=np.float32)
    ys = np.linspace(GRID_LO, GRID_HI, N_AXIS, dtype=np.float32)

    ac = np.empty((128, 192), np.float32)
    ac[:, 0:128] = xs[None, :]
    ac[:, 128:192] = X[:, 0].reshape(NCHUNK, 128).T
    yc = np.empty((128, 128), np.float16)
    yc[:, 0:64] = Y[:, 0].reshape(NCHUNK, 128).T
    yc[:, 64:128] = Y[:, 1].reshape(NCHUNK, 128).T

    x1t = X[:, 1].reshape(NCHUNK, 128).T
    in_maps = []
    for m in range(NCORES):
        bcm = np.empty((128, 80), np.float32)
        bcm[:, 0:64] = x1t
        bcm[:, 64:80] = ys[m * JS:(m + 1) * JS][None, :]
        in_maps.append({"bc": bcm, "ac": ac, "yc": yc})
    return in_maps


def run_on_cores(X, Y, **spmd_kwargs):
    """Run the SPMD kernel; returns BassKernelResults."""
    from concourse.bass_utils import run_bass_kernel_spmd

    nc = _get_program()
    in_maps = _host_inputs(X, Y)
    res = run_bass_kernel_spmd(nc, in_maps, core_ids=list(range(NCORES)),
                               **spmd_kwargs)
    return res


def kernel(X, Y):
    res = run_on_cores(X, Y)
    full = np.empty((3, N_AXIS, N_AXIS), dtype=np.float32)
    for m, r in enumerate(res.results):
        blk = r["out"]  # [128, 48] rows = i, cols = (c, j_local)
        for c in range(3):
            full[c, m * JS:(m + 1) * JS, :] = blk[:, c * JS:(c + 1) * JS].T
    return full



# revision 2
# speedup vs baseline: 1.3284x; 1.3284x over previous
"""ConvCNP encoder kernel for 8x TRN2 NeuronCores.

Math: the reference computes, for a 128x128 uniform grid g=(xs[i], ys[j]) and
n=8192 data points X (2-D) with values psi(Y) = [1, Y0, Y1]:

    Gram[g, x] = exp(-0.5*||g - X[x]||^2)
    fm = Gram @ psi                  # (G, 3); column 0 == row-sum (denominator)
    out[c, j, i] = fm[(i, j), c], with c=1,2 normalized by column 0.

The squared distance is separable over the grid axes:

    Gram[(i,j), x] = A[i, x] * B[j, x]
      A[i, x] = exp(-0.5*(xs[i] - X0[x])^2)     B[j, x] = exp(-0.5*(ys[j] - X1[x])^2)

so, with Bc = B * psi_c (row-wise):  fm[(i,j), c] = sum_x Bc[j, x] * A[i, x].

Sharding: the DATA-POINT axis x across the 8 cores — 1024 points per core,
grid replicated. Each core computes the partial (un-normalized) feature map
for the FULL grid over its x-slice; the host sums the 8 partials and
normalizes. This is 4.5x less exp work per core than grid-sharding (which
recomputes the full A on every core): (128+128)*1024 vs (128+16)*8192
Gram-factor elements.

Per core (k = 8 contraction chunks of 128 x-points):

    acc[i, (c, j)] = sum_k AT_k^T @ BfT_k      (PE, PSUM accum)
      AT_k  = exp(-0.5*(xs[i] - X0[x])^2)  in SBUF layout [x_part=128, i=128]
      BfT_k = [B | B*Y0 | B*Y1]            in SBUF layout [x_part=128, 384]

The sqdiff is a fused custom DVE op sq(Src0 - Src1) over broadcast APs; the
exp is one ACT pass per stripe (scale=-0.5). Gram factors are stored fp16
(fp32 argument keeps exp accuracy; fp16 feeds the PE at 1 cycle/column).
"""

import numpy as np
from contextlib import ExitStack

N_AXIS = 128          # grid points per axis
NPTS = 8192           # data points
NCORES = 8
XPC = NPTS // NCORES   # 1024 data points per core
NCHUNK = XPC // 128    # 8 contraction chunks of 128
HALF = NCHUNK // 2     # stripe size (chunks)
GRID_LO, GRID_HI = -2.0, 2.0

_CACHE = {}


def _register_sqdiff():
    """Register a fused (a-b)^2 custom DVE op (idempotent)."""
    from concourse import dve_ops
    from concourse.dve_spec import Spec, Src0, Src1, sq, lower
    from concourse.dve_uop import DveOpSpec

    name = "TENSOR_SQDIFF_X"
    for op in dve_ops.OPS:
        if op.name == name:
            return op
    spec = Spec(
        body=sq(Src0 - Src1),
        reference=lambda in0, in1, s0, s1, imm2: (in0.astype(np.float32) - in1) ** 2,
    )
    opcode = max(dve_ops._SUB_OPCODE_FOR_NAME.values()) + 1
    assert opcode < 0x20
    dve_ops._SUB_OPCODE_FOR_NAME[name] = opcode
    shas = {}
    for ver in ("v3", "v4"):
        s = DveOpSpec(name=name, opcode=opcode, uops=lower(spec, ver=ver), rd1_en=True)
        shas[ver] = s.sha(ver)
    op = dve_ops.DveOp(name, spec, subdim=False, uops_sha=shas)
    dve_ops.OPS.append(op)
    dve_ops.CUSTOM_DVE_SPECS[name] = spec
    return op


def _patch_walrus_flags():
    """Cap the compiler's semaphore file so the NEFF epilogue restores fewer
    semaphores (the restore is ~40ns/sem/engine of pure tail latency).
    Idempotent."""
    import concourse.bass_utils as bu

    if getattr(bu.run_command, "_sem_cap_patched", False):
        return
    orig = bu.run_command

    def run_command_capped(argv, **kwargs):
        if argv and "walrus_driver" in str(argv[0]) and any(
                str(a).startswith("--neff-output-filename") for a in argv):
            argv = list(argv) + ["--max-sem-num=176"]
        return orig(argv, **kwargs)

    run_command_capped._sem_cap_patched = True
    bu.run_command = run_command_capped


def _build_program():
    import concourse.bacc as bacc
    import concourse.mybir as mybir
    import concourse.tile as tile

    _patch_walrus_flags()
    sqdiff = _register_sqdiff()

    f32 = mybir.dt.float32
    f16 = mybir.dt.float16
    nc = bacc.Bacc("TRN2", target_bir_lowering=False, debug=False, num_devices=NCORES,
                   enable_partition_id=False, monotonic_sem_count=0)

    # Inputs (x-partition layout: partition = x within chunk, chunks on free):
    #   xc [128, 16] f32: X0 chunks (0:8)  | X1 chunks (8:16)   per-core slice
    #   yc [128, 16] f16: Y0 chunks (0:8)  | Y1 chunks (8:16)   per-core slice
    #   gc [128, 256] f32: xs replicated (0:128) | ys replicated (128:256)
    xc = nc.dram_tensor("xc", [128, 16], f32, kind="ExternalInput")
    yc = nc.dram_tensor("yc", [128, 16], f16, kind="ExternalInput")
    gc = nc.dram_tensor("gc", [128, 256], f32, kind="ExternalInput")
    out = nc.dram_tensor("out", [128, 3 * N_AXIS], f32, kind="ExternalOutput")

    with tile.TileContext(nc) as tc, ExitStack() as ctx:
        singles = ctx.enter_context(tc.tile_pool(name="singles", bufs=1))
        psum = ctx.enter_context(tc.tile_pool(name="psum", bufs=1, space="PSUM"))

        s_xc = singles.tile([128, 16], f32, tag="xc")
        nc.sync.dma_start(s_xc[:, :], xc[:, :])
        s_yc = singles.tile([128, 16], f16, tag="yc")
        nc.gpsimd.dma_start(s_yc[:, :], yc[:, :])
        s_gc = singles.tile([128, 256], f32, tag="gc")
        nc.scalar.dma_start(s_gc[:, :], gc[:, :])

        x0 = s_xc[:, 0:8]
        x1 = s_xc[:, 8:16]
        xsb = s_gc[:, 0:128]
        ysb = s_gc[:, 128:256]

        # Gram factor tiles: bf[x, k, c*128 + j], at[x, k, i]
        s_bsq = singles.tile([128, NCHUNK, 128], f32, tag="bsq")
        s_bf = singles.tile([128, NCHUNK, 3 * 128], f16, tag="bf")
        s_asq = singles.tile([128, NCHUNK, 128], f32, tag="asq")
        s_at = singles.tile([128, NCHUNK, 128], f16, tag="at")
        acc = psum.tile([128, 3 * N_AXIS], f32, tag="acc")

        for s in range(2):  # two stripes of HALF chunks for pipeline overlap
            ks = slice(s * HALF, (s + 1) * HALF)
            # B-side sqdiff + exp
            nc.vector._custom_dve(
                sqdiff,
                out=s_bsq[:, ks, :],
                in0=ysb.unsqueeze(1).broadcast_to([128, HALF, 128]),
                in1=x1[:, ks].unsqueeze(2).broadcast_to([128, HALF, 128]),
            )
            nc.scalar.activation(
                s_bf[:, ks, 0:128], s_bsq[:, ks, :],
                mybir.ActivationFunctionType.Exp, scale=-0.5,
            )
            # A-side sqdiff + exp
            nc.vector._custom_dve(
                sqdiff,
                out=s_asq[:, ks, :],
                in0=xsb.unsqueeze(1).broadcast_to([128, HALF, 128]),
                in1=x0[:, ks].unsqueeze(2).broadcast_to([128, HALF, 128]),
            )
            nc.scalar.activation(
                s_at[:, ks, :], s_asq[:, ks, :],
                mybir.ActivationFunctionType.Exp, scale=-0.5,
            )
            # B*Y0, B*Y1 (DVE, broadcast psi operand)
            for c in range(2):
                nc.vector.tensor_tensor(
                    s_bf[:, ks, (c + 1) * 128:(c + 2) * 128],
                    s_bf[:, ks, 0:128],
                    s_yc[:, c * NCHUNK:(c + 1) * NCHUNK][:, ks].unsqueeze(2)
                        .broadcast_to([128, HALF, 128]),
                    mybir.AluOpType.mult,
                )
            for k in range(s * HALF, (s + 1) * HALF):
                nc.tensor.matmul(
                    acc[:, :],
                    s_at[:, k, :],   # stationary lhsT: [128, 128] fp16
                    s_bf[:, k, :],   # moving rhs: [128, 384] fp16
                    start=(k == 0),
                    stop=(k == NCHUNK - 1),
                )

        s_out = singles.tile([128, 3 * N_AXIS], f32, tag="outt")
        nc.vector.tensor_copy(s_out[:, :], acc[:, :])
        nc.sync.dma_start(out[:, :], s_out[:, :])

    nc.finalize()
    return nc


def _get_program():
    if "nc" not in _CACHE:
        _CACHE["nc"] = _build_program()
    return _CACHE["nc"]


def _host_inputs(X, Y):
    """Build the per-core input maps (layout prep only)."""
    X = np.ascontiguousarray(np.asarray(X, dtype=np.float32))
    Y = np.ascontiguousarray(np.asarray(Y, dtype=np.float32))
    xs = np.linspace(GRID_LO, GRID_HI, N_AXIS, dtype=np.float32)
    ys = np.linspace(GRID_LO, GRID_HI, N_AXIS, dtype=np.float32)

    gc = np.empty((128, 256), np.float32)
    gc[:, 0:128] = xs[None, :]
    gc[:, 128:256] = ys[None, :]

    in_maps = []
    for m in range(NCORES):
        sl = slice(m * XPC, (m + 1) * XPC)
        xcm = np.empty((128, 16), np.float32)
        xcm[:, 0:8] = X[sl, 0].reshape(NCHUNK, 128).T
        xcm[:, 8:16] = X[sl, 1].reshape(NCHUNK, 128).T
        ycm = np.empty((128, 16), np.float16)
        ycm[:, 0:8] = Y[sl, 0].reshape(NCHUNK, 128).T
        ycm[:, 8:16] = Y[sl, 1].reshape(NCHUNK, 128).T
        in_maps.append({"xc": xcm, "yc": ycm, "gc": gc})
    return in_maps


def run_on_cores(X, Y, **spmd_kwargs):
    """Run the SPMD kernel; returns BassKernelResults."""
    from concourse.bass_utils import run_bass_kernel_spmd

    nc = _get_program()
    in_maps = _host_inputs(X, Y)
    res = run_bass_kernel_spmd(nc, in_maps, core_ids=list(range(NCORES)),
                               **spmd_kwargs)
    return res


def kernel(X, Y):
    res = run_on_cores(X, Y)
    # Sum the per-core partial feature maps, then normalize.
    acc = np.zeros((128, 3 * N_AXIS), np.float64)
    for r in res.results:
        acc += r["out"]
    fm = acc.reshape(128, 3, N_AXIS)        # [i, c, j]
    full = fm.transpose(1, 2, 0).astype(np.float32)  # [c, j, i]
    full[1] /= full[0]
    full[2] /= full[0]
    return np.ascontiguousarray(full)


# revision 3
# speedup vs baseline: 1.3508x; 1.0169x over previous
"""ConvCNP encoder kernel for 8x TRN2 NeuronCores.

Math: the reference computes, for a 128x128 uniform grid g=(xs[i], ys[j]) and
n=8192 data points X (2-D) with values psi(Y) = [1, Y0, Y1]:

    Gram[g, x] = exp(-0.5*||g - X[x]||^2)
    fm = Gram @ psi                  # (G, 3); column 0 == row-sum (denominator)
    out[c, j, i] = fm[(i, j), c], with c=1,2 normalized by column 0.

The squared distance is separable over the grid axes (xs == ys == the same
128-point linspace g):

    Gram[(i,j), x] = A[i, x] * B[j, x]
      A[i, x] = exp(-0.5*(g[i] - X0[x])^2)     B[j, x] = exp(-0.5*(g[j] - X1[x])^2)

so, with Bc = B * psi_c (row-wise):  fm[(i,j), c] = sum_x Bc[j, x] * A[i, x].

Sharding: the DATA-POINT axis x across the 8 cores — 1024 points per core,
grid replicated. Each core computes the partial (un-normalized) feature map
for the FULL grid over its x-slice; the host sums the 8 partials and
normalizes. This is 4.5x less exp work per core than grid-sharding (which
recomputes the full A on every core): (128+128)*1024 vs (128+16)*8192
Gram-factor elements.

Per core (8 contraction chunks of 128 x-points, striped 2 at a time):

    acc[i, (c, j)] = sum_k AT_k^T @ BfT_k      (PE, PSUM accum)
      AT_k  = exp(-0.5*(g[i] - X0[x])^2)   in SBUF layout [x_part=128, i=128]
      BfT_k = [B | B*Y0 | B*Y1]            in SBUF layout [x_part=128, 384]

Engine split per stripe: DVE does the two sqdiffs (fused custom op
sq(Src0-Src1) over broadcast APs), ACT the two exps (scale=-0.5), the Pool
engine the psi-muls (one op over an interleaved (k,c) broadcast AP) and the
grid-vector generation (iota + affine — no grid DMA at all). Gram factors
are fp16 (fp32 exp argument keeps accuracy; fp16 feeds the PE at
1 cycle/column). Output is the raw fp16 partial [128, 3*128]; the epilogue
splits the PSUM->SBUF copy across Vector/ACT and the store across two DMA
queues.
"""

import numpy as np
from contextlib import ExitStack

N_AXIS = 128          # grid points per axis
NPTS = 8192           # data points
NCORES = 8
XPC = NPTS // NCORES   # 1024 data points per core
NCHUNK = XPC // 128    # 8 contraction chunks of 128
SW = 2                 # chunks per stripe
NSTRIPE = NCHUNK // SW
GRID_LO, GRID_HI = -2.0, 2.0
MAX_SEM = 32           # walrus semaphore cap (restore is ~40ns/sem/engine)

_CACHE = {}


def _register_sqdiff():
    """Register a fused (a-b)^2 custom DVE op (idempotent)."""
    from concourse import dve_ops
    from concourse.dve_spec import Spec, Src0, Src1, sq, lower
    from concourse.dve_uop import DveOpSpec

    name = "TENSOR_SQDIFF_X"
    for op in dve_ops.OPS:
        if op.name == name:
            return op
    spec = Spec(
        body=sq(Src0 - Src1),
        reference=lambda in0, in1, s0, s1, imm2: (in0.astype(np.float32) - in1) ** 2,
    )
    opcode = max(dve_ops._SUB_OPCODE_FOR_NAME.values()) + 1
    assert opcode < 0x20
    dve_ops._SUB_OPCODE_FOR_NAME[name] = opcode
    shas = {}
    for ver in ("v3", "v4"):
        s = DveOpSpec(name=name, opcode=opcode, uops=lower(spec, ver=ver), rd1_en=True)
        shas[ver] = s.sha(ver)
    op = dve_ops.DveOp(name, spec, subdim=False, uops_sha=shas)
    dve_ops.OPS.append(op)
    dve_ops.CUSTOM_DVE_SPECS[name] = spec
    return op


def _patch_walrus_flags():
    """Cap the compiler's semaphore file so the NEFF prologue/epilogue
    save/restore covers MAX_SEM semaphores instead of all 254 (the restore
    is ~40ns/sem/engine of pure tail latency). Idempotent."""
    import concourse.bass_utils as bu

    if getattr(bu.run_command, "_sem_cap_patched", False):
        return
    orig = bu.run_command

    def run_command_capped(argv, **kwargs):
        if argv and "walrus_driver" in str(argv[0]) and any(
                str(a).startswith("--neff-output-filename") for a in argv):
            argv = list(argv) + [f"--max-sem-num={MAX_SEM}"]
        return orig(argv, **kwargs)

    run_command_capped._sem_cap_patched = True
    bu.run_command = run_command_capped


def _build_program():
    import concourse.bacc as bacc
    import concourse.mybir as mybir
    import concourse.tile as tile

    _patch_walrus_flags()
    sqdiff = _register_sqdiff()

    f32 = mybir.dt.float32
    f16 = mybir.dt.float16
    nc = bacc.Bacc("TRN2", target_bir_lowering=False, debug=False, num_devices=NCORES,
                   enable_partition_id=False, monotonic_sem_count=0)

    # Inputs (x-partition layout: partition = x within chunk, chunks on free):
    #   xc [128, 16] f32: X0 chunks (0:8) | X1 chunks (8:16)        per-core
    #   yc [128, 16] f16: interleaved col 2k = Y0 chunk k, 2k+1 = Y1 chunk k
    xc = nc.dram_tensor("xc", [128, 16], f32, kind="ExternalInput")
    yc = nc.dram_tensor("yc", [128, 16], f16, kind="ExternalInput")
    out = nc.dram_tensor("out", [128, 3 * N_AXIS], f16, kind="ExternalOutput")

    with tile.TileContext(nc) as tc, ExitStack() as ctx:
        singles = ctx.enter_context(tc.tile_pool(name="singles", bufs=1))
        psum = ctx.enter_context(tc.tile_pool(name="psum", bufs=1, space="PSUM"))

        s_xc = singles.tile([128, 16], f32, tag="xc")
        nc.sync.dma_start(s_xc[:, :], xc[:, :])
        s_yc = singles.tile([128, 16], f16, tag="yc")
        nc.sync.dma_start(s_yc[:, :], yc[:, :])

        # Grid vector, generated on-device: g[p, i] = -2 + i*(4/127).
        s_gi = singles.tile([128, 128], f32, tag="gi")
        nc.gpsimd.iota(s_gi[:, :], [[1, 128]], channel_multiplier=0,
                       allow_small_or_imprecise_dtypes=True)
        s_gv = singles.tile([128, 128], f32, tag="gv")
        nc.gpsimd.tensor_scalar(
            out=s_gv[:, :], in0=s_gi[:, :],
            scalar1=(GRID_HI - GRID_LO) / (N_AXIS - 1), scalar2=GRID_LO,
            op0=mybir.AluOpType.mult, op1=mybir.AluOpType.add,
        )

        x0 = s_xc[:, 0:8]
        x1 = s_xc[:, 8:16]

        # Gram factor tiles: bf[x, k, c*128 + j], at[x, k, i]
        s_bsq = singles.tile([128, NCHUNK, 128], f32, tag="bsq")
        s_bf = singles.tile([128, NCHUNK, 3 * 128], f16, tag="bf")
        s_asq = singles.tile([128, NCHUNK, 128], f32, tag="asq")
        s_at = singles.tile([128, NCHUNK, 128], f16, tag="at")
        acc = psum.tile([128, 3 * N_AXIS], f32, tag="acc")

        for s in range(NSTRIPE):
            ks = slice(s * SW, (s + 1) * SW)
            # B-side sqdiff + exp
            nc.vector._custom_dve(
                sqdiff,
                out=s_bsq[:, ks, :],
                in0=s_gv.unsqueeze(1).broadcast_to([128, SW, 128]),
                in1=x1[:, ks].unsqueeze(2).broadcast_to([128, SW, 128]),
            )
            nc.scalar.activation(
                s_bf[:, ks, 0:128], s_bsq[:, ks, :],
                mybir.ActivationFunctionType.Exp, scale=-0.5,
            )
            # A-side sqdiff + exp
            nc.vector._custom_dve(
                sqdiff,
                out=s_asq[:, ks, :],
                in0=s_gv.unsqueeze(1).broadcast_to([128, SW, 128]),
                in1=x0[:, ks].unsqueeze(2).broadcast_to([128, SW, 128]),
            )
            nc.scalar.activation(
                s_at[:, ks, :], s_asq[:, ks, :],
                mybir.ActivationFunctionType.Exp, scale=-0.5,
            )
            # B*Y0, B*Y1 in one Pool-engine pass over the interleaved (k, c)
            # broadcast AP.
            nc.gpsimd.tensor_tensor(
                s_bf[:, ks, 128:384].rearrange("p k (c j) -> p k c j", c=2),
                s_bf[:, ks, 0:128].unsqueeze(2).broadcast_to([128, SW, 2, 128]),
                s_yc[:, 2 * s * SW:2 * (s + 1) * SW]
                    .rearrange("p (k c) -> p k c", c=2)
                    .unsqueeze(3).broadcast_to([128, SW, 2, 128]),
                mybir.AluOpType.mult,
            )
            for k in range(s * SW, (s + 1) * SW):
                nc.tensor.matmul(
                    acc[:, :],
                    s_at[:, k, :],   # stationary lhsT: [128, 128] fp16
                    s_bf[:, k, :],   # moving rhs: [128, 384] fp16
                    start=(k == 0),
                    stop=(k == NCHUNK - 1),
                )

        # Epilogue: PSUM -> SBUF fp16 split across Vector/ACT, store split
        # across two DMA queues.
        s_out = singles.tile([128, 3 * N_AXIS], f16, tag="outt")
        H = 3 * N_AXIS // 2
        nc.vector.tensor_copy(s_out[:, 0:H], acc[:, 0:H])
        nc.scalar.activation(s_out[:, H:], acc[:, H:],
                             mybir.ActivationFunctionType.Copy)
        nc.sync.dma_start(out[:, 0:H], s_out[:, 0:H])
        nc.scalar.dma_start(out[:, H:], s_out[:, H:])

    nc.finalize()
    return nc


def _get_program():
    if "nc" not in _CACHE:
        _CACHE["nc"] = _build_program()
    return _CACHE["nc"]


def _host_inputs(X, Y):
    """Build the per-core input maps (layout prep only)."""
    X = np.ascontiguousarray(np.asarray(X, dtype=np.float32))
    Y = np.ascontiguousarray(np.asarray(Y, dtype=np.float32))

    in_maps = []
    for m in range(NCORES):
        sl = slice(m * XPC, (m + 1) * XPC)
        xcm = np.empty((128, 16), np.float32)
        xcm[:, 0:8] = X[sl, 0].reshape(NCHUNK, 128).T
        xcm[:, 8:16] = X[sl, 1].reshape(NCHUNK, 128).T
        ycm = np.empty((128, 16), np.float16)
        ycm[:, 0::2] = Y[sl, 0].reshape(NCHUNK, 128).T
        ycm[:, 1::2] = Y[sl, 1].reshape(NCHUNK, 128).T
        in_maps.append({"xc": xcm, "yc": ycm})
    return in_maps


def run_on_cores(X, Y, **spmd_kwargs):
    """Run the SPMD kernel; returns BassKernelResults."""
    from concourse.bass_utils import run_bass_kernel_spmd

    nc = _get_program()
    in_maps = _host_inputs(X, Y)
    res = run_bass_kernel_spmd(nc, in_maps, core_ids=list(range(NCORES)),
                               **spmd_kwargs)
    return res


def kernel(X, Y):
    res = run_on_cores(X, Y)
    # Sum the per-core partial feature maps, then normalize.
    acc = np.zeros((128, 3 * N_AXIS), np.float64)
    for r in res.results:
        acc += r["out"]
    fm = acc.reshape(128, 3, N_AXIS)                 # [i, c, j]
    full = fm.transpose(1, 2, 0).astype(np.float32)  # [c, j, i]
    full[1] /= full[0]
    full[2] /= full[0]
    return np.ascontiguousarray(full)
